# revision 9
# baseline (speedup 1.0000x reference)
"""PoseVelGraph residuals on 8 Trainium2 NeuronCores.

Strategy (see sharding_hint): shard edges/poses data-parallel across the 8
cores; each edge shard carries its endpoint node features (edge-cut GNN
distribution, host attaches nodes[edges[:,0]]/nodes[edges[:,1]] during
sharding).  The [M,*] IMU chain shards along the node axis; the 1-element
halo for diff() is handled by shipping row r and row r+1 slabs.

On-device layout: everything is SoA "component blocks" per partition —
a tile [128, C*L] holds C component blocks of L contiguous elements, so all
quaternion/cross-product algebra runs as full-width [128, c*L] DVE slab ops
(fp16, 2x packing).  The transcendental chain of SO3 log (sqrt/atan/recip)
runs in fp32 on ACT + DVE custom reciprocal.

Residuals computed per edge e (pgerr) and per chain row r:
  pgerr     = se3_log( poses^-1 o n1^-1 o n2 )              [E,6]
  adjvelerr = 0.1 * (imu_dvels - (vels[1:] - vels[:-1]))    [M,3]
  imuroterr = so3_log( drot^-1 o q[:-1]^-1 o q[1:] )        [M,3]
  transvelerr = 0.1 * ((t[1:]-t[:-1]) - (vels[:-1]*dts + imu_dtrans))
Output = concat of the four raveled blocks.
"""

import numpy as np

import concourse.bass as bass
import concourse.mybir as mybir
from concourse.tile import TileContext

F16 = mybir.dt.float16
F32 = mybir.dt.float32
OP = mybir.AluOpType
AF = mybir.ActivationFunctionType
P = 128
PI = float(np.pi)
EPS = 1e-8

LAST_RESULT = None  # BassKernelResults of the most recent run (for test harness)

# full-problem config
FULL = dict(E=2_000_000, N=1_000_000, M=999_999, K=490, NT=4, NS=2, NC=8)


def _split_excess_waits(nc, max_waits=1):
    """walrus CoreV3 codegen rejects instructions carrying several sem waits
    ("Too many sync wait commands").  Hoist excess waits onto same-engine
    NoOps placed just before the instruction; per-engine in-order execution
    makes this equivalent."""
    for f in nc.m.functions:
        for bb in f.blocks:
            new = []
            for ins in bb.instructions:
                si = ins.sync_info
                w = list(si.on_wait) if si and si.on_wait else []
                if len(w) > max_waits:
                    keep = w[-max_waits:]
                    extras = w[:-max_waits]
                    for i in range(0, len(extras), max_waits):
                        chunk = extras[i:i + max_waits]
                        nop = mybir.InstNoOp(
                            name=f"I-waitfix-{nc.next_id()}",
                            engine=ins.engine,
                            sync_info=mybir.SyncInfo(on_wait=chunk, on_update=[]),
                        )
                        new.append(nop)
                    si.on_wait = keep
                new.append(ins)
            bb.instructions[:] = new


class _Emit:
    """Slab-op emitter.  All tiles are [128, c*L] fp16 component blocks unless
    noted; the fp32 scalar chain uses [128, L] fp32 tiles."""

    def __init__(self, nc, tc, pools, L):
        self.nc = nc
        self.tc = tc
        self.pools = pools
        self.L = L
        self.V = nc.vector
        self.A = nc.scalar
        self.S = nc.sync
        self._uid = 0

    # ---- tile allocation helpers (shared rotating tags) ----
    def _t(self, pool, shape, dt, tag):
        self._uid += 1
        return self.pools[pool].tile(shape, dt, name=f"{tag}_{self._uid}", tag=tag)

    def d6(self):
        return self._t("d6", [P, 6 * self.L], F16, "d6")

    def s3(self):
        return self._t("s3", [P, 3 * self.L], F16, "s3")

    def s1(self):
        return self._t("s1", [P, self.L], F16, "s1")

    def sk(self):
        return self._t("sk", [P, self.L], F32, "sk")

    def ski(self):
        # int32 masks (CopyPredicated requires an integer mask dtype)
        return self._t("sk", [P, self.L], mybir.dt.int32, "sk")

    # block views of a [128, C*L] tile AP
    def blk(self, ap, i, n=1):
        return ap[:, i * self.L:(i + n) * self.L]

    def dup(self, d6t):
        # duplicate the first 3 blocks into blocks 3..5 (for cross-product
        # "rotated" views)
        self.A.copy(self.blk(d6t, 3, 3), self.blk(d6t, 0, 3))

    def bcast3(self, out3, src1):
        # out3[c] = src1 for c in 0..2  (broadcast a scalar-slab to 3 blocks)
        for c in range(3):
            self.A.copy(self.blk(out3, c), src1)

    # ---- math emitters ----
    def qmul_conj(self, vA6, wA3, wAk, vB6, wB3, wBk, out_v3, out_wk):
        """out = conj(A) (x) B.
        out_w = wA*wB + dot(vA,vB);  out_v = wA*vB - wB*vA - vA x vB.
        vA6/vB6 are dup6 tiles; wA3/wB3 are 3-block broadcasts; wAk/wBk
        scalar-slabs."""
        V, b = self.V, self.blk
        pv = self.s3()
        V.tensor_tensor(out=pv[:], in0=b(vA6, 0, 3), in1=b(vB6, 0, 3), op=OP.mult)
        s1a = self.s1()
        V.tensor_tensor(out=s1a[:], in0=b(pv, 0), in1=b(pv, 1), op=OP.add)
        pw = self.s1()
        V.tensor_tensor(out=pw[:], in0=wAk, in1=wBk, op=OP.mult)
        V.tensor_tensor(out=s1a[:], in0=s1a[:], in1=b(pv, 2), op=OP.add)
        V.tensor_tensor(out=out_wk, in0=s1a[:], in1=pw[:], op=OP.add)
        o2 = self.s3()
        V.tensor_tensor(out=out_v3, in0=wA3[:], in1=b(vB6, 0, 3), op=OP.mult)
        V.tensor_tensor(out=o2[:], in0=wB3[:], in1=b(vA6, 0, 3), op=OP.mult)
        V.tensor_tensor(out=out_v3, in0=out_v3, in1=o2[:], op=OP.subtract)
        m1 = self.s3()
        V.tensor_tensor(out=m1[:], in0=b(vA6, 1, 3), in1=b(vB6, 2, 3), op=OP.mult)
        V.tensor_tensor(out=out_v3, in0=out_v3, in1=m1[:], op=OP.subtract)
        m2 = self.s3()
        V.tensor_tensor(out=m2[:], in0=b(vA6, 2, 3), in1=b(vB6, 1, 3), op=OP.mult)
        V.tensor_tensor(out=out_v3, in0=out_v3, in1=m2[:], op=OP.add)

    def cross(self, a6, b6, out3):
        """out = a x b using rotated views of dup6 tiles."""
        V, b = self.V, self.blk
        t2 = self.s3()
        V.tensor_tensor(out=out3, in0=b(a6, 1, 3), in1=b(b6, 2, 3), op=OP.mult)
        V.tensor_tensor(out=t2[:], in0=b(a6, 2, 3), in1=b(b6, 1, 3), op=OP.mult)
        V.tensor_tensor(out=out3, in0=out3, in1=t2[:], op=OP.subtract)

    def qrot_conj(self, v6, w3, u6, out3):
        """out = R(conj(q))*u = u - 2w*(v x u) + 2*(v x (v x u))."""
        V, b = self.V, self.blk
        c1d6 = self.d6()
        self.cross(v6, u6, b(c1d6, 0, 3))
        self.dup(c1d6)
        c2 = self.s3()
        self.cross(v6, c1d6, c2[:])
        mw = self.s3()
        V.tensor_tensor(out=mw[:], in0=w3[:], in1=b(c1d6, 0, 3), op=OP.mult)
        V.scalar_tensor_tensor(out=out3, in0=mw[:], scalar=-2.0, in1=b(u6, 0, 3),
                               op0=OP.mult, op1=OP.add)
        V.scalar_tensor_tensor(out=out3, in0=c2[:], scalar=2.0, in1=out3,
                               op0=OP.mult, op1=OP.add)

    def so3_se3(self, qv3, qwk, out_phi3, te6=None, out_tau3=None):
        """phi = so3_log(q);  optionally tau = V^-1 t of se3_log (te6 given).
        Writes phi (and tau) as fp16 3-block slabs.  Transcendental chain in
        fp32.  atan2 via the swap trick keeps |atan arg| <= 1 (HW table range).
        For a unit q: cos(th)= 2w^2-1, sin(th)=2nw, so the V^-1 coefficient
        1/th^2 - (1+cos)/(2 th sin) reduces to 1/th^2 - w/(2 th n)."""
        V, A, b = self.V, self.A, self.blk
        sqx, sqy, sqz = self.sk(), self.sk(), self.sk()
        A.activation(out=sqx[:], in_=qv3[:, 0:self.L], func=AF.Square)
        A.activation(out=sqy[:], in_=qv3[:, self.L:2 * self.L], func=AF.Square)
        A.activation(out=sqz[:], in_=qv3[:, 2 * self.L:3 * self.L], func=AF.Square)
        n2 = self.sk()
        V.tensor_tensor(out=n2[:], in0=sqx[:], in1=sqy[:], op=OP.add)
        V.tensor_tensor(out=n2[:], in0=n2[:], in1=sqz[:], op=OP.add)
        n_ = self.sk()
        A.activation(out=n_[:], in_=n2[:], func=AF.Sqrt)
        w32 = self.sk()
        A.copy(out=w32[:], in_=qwk)
        nG = self.sk()
        V.tensor_scalar(out=nG[:], in0=n_[:], scalar1=1e-20, scalar2=None, op0=OP.max)
        rn = self.sk()
        V.reciprocal(out=rn[:], in_=nG[:])
        rw = self.sk()
        V.reciprocal(out=rw[:], in_=w32[:])
        aw = self.sk()
        A.activation(out=aw[:], in_=w32[:], func=AF.Abs)
        m1 = self.ski()
        V.tensor_tensor(out=m1[:], in0=n_[:], in1=aw[:], op=OP.is_ge)
        a1 = self.sk()
        V.tensor_tensor(out=a1[:], in0=w32[:], in1=rn[:], op=OP.mult)
        a2 = self.sk()
        V.tensor_tensor(out=a2[:], in0=n_[:], in1=rw[:], op=OP.mult)
        arg = self.sk()
        V.select(out=arg[:], mask=m1[:], on_true=a1[:], on_false=a2[:])
        atA = self.sk()
        A.activation(out=atA[:], in_=arg[:], func=AF.Arctan)
        # theta = select(n>=|w|, pi - 2*atan(w/n), 2*atan(n/w) + 2*pi*[w<0])
        thA = self.sk()
        V.tensor_scalar(out=thA[:], in0=atA[:], scalar1=-2.0, scalar2=PI,
                        op0=OP.mult, op1=OP.add)
        at2 = self.sk()
        V.tensor_scalar(out=at2[:], in0=atA[:], scalar1=2.0, scalar2=None,
                        op0=OP.mult)
        ngm = self.sk()
        V.tensor_scalar(out=ngm[:], in0=w32[:], scalar1=0.0, scalar2=None, op0=OP.is_lt)
        thB = self.sk()
        V.scalar_tensor_tensor(out=thB[:], in0=ngm[:], scalar=2.0 * PI, in1=at2[:],
                               op0=OP.mult, op1=OP.add)
        th = self.sk()
        V.select(out=th[:], mask=m1[:], on_true=thA[:], on_false=thB[:])
        kb = self.sk()
        V.tensor_tensor(out=kb[:], in0=th[:], in1=rn[:], op=OP.mult)
        ks = self.sk()
        V.tensor_scalar(out=ks[:], in0=rw[:], scalar1=2.0, scalar2=None, op0=OP.mult)
        mn = self.ski()
        V.tensor_scalar(out=mn[:], in0=n_[:], scalar1=EPS, scalar2=None, op0=OP.is_gt)
        kk = self.sk()
        V.select(out=kk[:], mask=mn[:], on_true=kb[:], on_false=ks[:])
        kk3 = self.s3()
        self.bcast3(kk3, kk[:])
        V.tensor_tensor(out=out_phi3, in0=qv3[:], in1=kk3[:], op=OP.mult)
        if te6 is None:
            return
        # ---- tau = t - 0.5 p x t + coef * (p x (p x t)) ----
        thG = self.sk()
        V.tensor_scalar(out=thG[:], in0=th[:], scalar1=1e-6, scalar2=None, op0=OP.max)
        k1 = self.sk()
        V.reciprocal(out=k1[:], in_=thG[:])
        dd = self.sk()
        V.scalar_tensor_tensor(out=dd[:], in0=a1[:], scalar=-0.5, in1=k1[:],
                               op0=OP.mult, op1=OP.add)
        coefb = self.sk()
        V.tensor_tensor(out=coefb[:], in0=dd[:], in1=k1[:], op=OP.mult)
        # small-theta branch: coef -> 1/12 (exact cancellation keeps it finite)
        ms = self.sk()
        V.tensor_scalar(out=ms[:], in0=th[:], scalar1=1e-4, scalar2=None, op0=OP.is_lt)
        u1 = self.sk()
        V.scalar_tensor_tensor(out=u1[:], in0=coefb[:], scalar=-1.0, in1=ms[:],
                               op0=OP.mult, op1=OP.mult)
        u2 = self.sk()
        V.scalar_tensor_tensor(out=u2[:], in0=ms[:], scalar=1.0 / 12.0, in1=coefb[:],
                               op0=OP.mult, op1=OP.add)
        coef = self.sk()
        V.tensor_tensor(out=coef[:], in0=u1[:], in1=u2[:], op=OP.add)
        coef3 = self.s3()
        self.bcast3(coef3, coef[:])
        phi6 = self.d6()
        A.copy(out=b(phi6, 0, 3), in_=out_phi3)
        self.dup(phi6)
        px6 = self.d6()
        self.cross(phi6, te6, b(px6, 0, 3))
        self.dup(px6)
        cpp = self.s3()
        self.cross(phi6, px6, cpp[:])
        gg = self.s3()
        V.scalar_tensor_tensor(out=gg[:], in0=b(px6, 0, 3), scalar=-0.5,
                               in1=b(te6, 0, 3), op0=OP.mult, op1=OP.add)
        hh = self.s3()
        V.tensor_tensor(out=hh[:], in0=coef3[:], in1=cpp[:], op=OP.mult)
        V.tensor_tensor(out=out_tau3, in0=gg[:], in1=hh[:], op=OP.add)


def build_nc(cfg):
    K, NT, NS = cfg["K"], cfg["NT"], cfg["NS"]
    L = K
    nc = bass.Bass()
    dram_in = {}
    for nm, sh in [
        ("en1", [NT, P, 7 * K]), ("en2", [NT, P, 7 * K]), ("eps", [NT, P, 7 * K]),
        ("in0", [NS, P, 7 * K]), ("in1", [NS, P, 7 * K]),
        ("iv0", [NS, P, 3 * K]), ("iv1", [NS, P, 3 * K]),
        ("idr", [NS, P, 4 * K]), ("idtr", [NS, P, 3 * K]), ("idv", [NS, P, 3 * K]),
        ("idts", [NS, P, K]),
    ]:
        dram_in[nm] = nc.dram_tensor(nm, sh, F16, kind="ExternalInput")
    pg = nc.dram_tensor("pg", [NT, P, 6 * K], F16, kind="ExternalOutput")
    adj = nc.dram_tensor("adj", [NS, P, 3 * K], F16, kind="ExternalOutput")
    rot = nc.dram_tensor("rot", [NS, P, 3 * K], F16, kind="ExternalOutput")
    tvl = nc.dram_tensor("tvl", [NS, P, 3 * K], F16, kind="ExternalOutput")

    with TileContext(nc) as tc:
        with (
            tc.tile_pool(name="io_e", bufs=2) as io_e,
            tc.tile_pool(name="io_i", bufs=1) as io_i,
            tc.tile_pool(name="d6", bufs=6) as d6p,
            tc.tile_pool(name="s3", bufs=10) as s3p,
            tc.tile_pool(name="s1", bufs=6) as s1p,
            tc.tile_pool(name="sk", bufs=14) as skp,
        ):
            pools = dict(d6=d6p, s3=s3p, s1=s1p, sk=skp)
            em = _Emit(nc, tc, pools, L)
            V, A, S, b = em.V, em.A, em.S, em.blk

            def edge_tile(t):
                a1 = io_e.tile([P, 7 * K], F16, name=f"a1_{t}", tag="a1")
                a2 = io_e.tile([P, 7 * K], F16, name=f"a2_{t}", tag="a2")
                ps = io_e.tile([P, 7 * K], F16, name=f"ps_{t}", tag="ps")
                ot = io_e.tile([P, 6 * K], F16, name=f"ot_{t}", tag="ot")
                S.dma_start(out=a1[:], in_=dram_in["en1"][t, :, :])
                S.dma_start(out=a2[:], in_=dram_in["en2"][t, :, :])
                S.dma_start(out=ps[:], in_=dram_in["eps"][t, :, :])
                # materialize dup6/broadcast helpers
                v1d6 = em.d6()
                A.copy(out=b(v1d6, 0, 3), in_=b(a1[:], 3, 3))
                em.dup(v1d6)
                v2d6 = em.d6()
                A.copy(out=b(v2d6, 0, 3), in_=b(a2[:], 3, 3))
                em.dup(v2d6)
                w13 = em.s3()
                em.bcast3(w13, b(a1[:], 6))
                w23 = em.s3()
                em.bcast3(w23, b(a2[:], 6))
                # qa = conj(q1) (x) q2
                qa6 = em.d6()
                qaw = em.s1()
                em.qmul_conj(v1d6, w13, b(a1[:], 6), v2d6, w23, b(a2[:], 6),
                             b(qa6, 0, 3), qaw[:])
                em.dup(qa6)
                # qe = conj(qp) (x) qa
                qp6 = em.d6()
                A.copy(out=b(qp6, 0, 3), in_=b(ps[:], 3, 3))
                em.dup(qp6)
                wp3 = em.s3()
                em.bcast3(wp3, b(ps[:], 6))
                wa3 = em.s3()
                em.bcast3(wa3, qaw[:])
                qev = em.s3()
                qew = em.s1()
                em.qmul_conj(qp6, wp3, b(ps[:], 6), qa6, wa3, qaw[:],
                             qev[:], qew[:])
                # u = t2 - t1 ; ta = R(conj(q1)) u
                u6 = em.d6()
                V.tensor_tensor(out=b(u6, 0, 3), in0=b(a2[:], 0, 3),
                                in1=b(a1[:], 0, 3), op=OP.subtract)
                em.dup(u6)
                ta3 = em.s3()
                em.qrot_conj(v1d6, w13, u6, ta3[:])
                # v' = ta - tp ; te = R(conj(qp)) v'
                vp6 = em.d6()
                V.tensor_tensor(out=b(vp6, 0, 3), in0=ta3[:], in1=b(ps[:], 0, 3),
                                op=OP.subtract)
                em.dup(vp6)
                te6 = em.d6()
                em.qrot_conj(qp6, wp3, vp6, b(te6, 0, 3))
                em.dup(te6)
                # se3 log -> out tile (tau in blocks 0..2, phi in blocks 3..5)
                em.so3_se3(qev[:], qew[:], b(ot[:], 3, 3), te6, b(ot[:], 0, 3))
                S.dma_start(out=pg[t, :, :], in_=ot[:])

            def imu_tile(s):
                tin0 = io_i.tile([P, 7 * K], F16, name=f"tin0_{s}", tag="tin0")
                tin1 = io_i.tile([P, 7 * K], F16, name=f"tin1_{s}", tag="tin1")
                tv0 = io_i.tile([P, 3 * K], F16, name=f"tv0_{s}", tag="tv0")
                tv1 = io_i.tile([P, 3 * K], F16, name=f"tv1_{s}", tag="tv1")
                tdr = io_i.tile([P, 4 * K], F16, name=f"tdr_{s}", tag="tdr")
                tdtr = io_i.tile([P, 3 * K], F16, name=f"tdtr_{s}", tag="tdtr")
                tdv = io_i.tile([P, 3 * K], F16, name=f"tdv_{s}", tag="tdv")
                tdts = io_i.tile([P, K], F16, name=f"tdts_{s}", tag="tdts")
                to_a = io_i.tile([P, 3 * K], F16, name=f"to_a_{s}", tag="to_a")
                to_r = io_i.tile([P, 3 * K], F16, name=f"to_r_{s}", tag="to_r")
                to_t = io_i.tile([P, 3 * K], F16, name=f"to_t_{s}", tag="to_t")
                for tile_, nm in [(tin0, "in0"), (tin1, "in1"), (tv0, "iv0"),
                                  (tv1, "iv1"), (tdr, "idr"), (tdtr, "idtr"),
                                  (tdv, "idv"), (tdts, "idts")]:
                    S.dma_start(out=tile_[:], in_=dram_in[nm][s, :, :])
                # part 2: adj = 0.1*(dv - (v1 - v0))
                dvv = em.s3()
                V.tensor_tensor(out=dvv[:], in0=tv1[:], in1=tv0[:], op=OP.subtract)
                ee = em.s3()
                V.tensor_tensor(out=ee[:], in0=tdv[:], in1=dvv[:], op=OP.subtract)
                A.mul(out=to_a[:], in_=ee[:], mul=0.1)
                S.dma_start(out=adj[s, :, :], in_=to_a[:])
                # part 4: tvl = 0.1*((t1 - t0) - (v0*dts + dtr))
                dts3 = em.s3()
                em.bcast3(dts3, tdts[:])
                y1 = em.s3()
                V.tensor_tensor(out=y1[:], in0=tv0[:], in1=dts3[:], op=OP.mult)
                V.tensor_tensor(out=y1[:], in0=y1[:], in1=tdtr[:], op=OP.add)
                y3 = em.s3()
                V.tensor_tensor(out=y3[:], in0=b(tin1[:], 0, 3),
                                in1=b(tin0[:], 0, 3), op=OP.subtract)
                V.tensor_tensor(out=y3[:], in0=y3[:], in1=y1[:], op=OP.subtract)
                A.mul(out=to_t[:], in_=y3[:], mul=0.1)
                S.dma_start(out=tvl[s, :, :], in_=to_t[:])
                # part 3: qre = conj(dr) (x) (conj(q0) (x) q1) ; rot = so3_log
                v06 = em.d6()
                A.copy(out=b(v06, 0, 3), in_=b(tin0[:], 3, 3))
                em.dup(v06)
                v16 = em.d6()
                A.copy(out=b(v16, 0, 3), in_=b(tin1[:], 3, 3))
                em.dup(v16)
                w03 = em.s3()
                em.bcast3(w03, b(tin0[:], 6))
                w13b = em.s3()
                em.bcast3(w13b, b(tin1[:], 6))
                qq6 = em.d6()
                qqw = em.s1()
                em.qmul_conj(v06, w03, b(tin0[:], 6), v16, w13b, b(tin1[:], 6),
                             b(qq6, 0, 3), qqw[:])
                em.dup(qq6)
                dr6 = em.d6()
                A.copy(out=b(dr6, 0, 3), in_=tdr[:, 0:3 * K])
                em.dup(dr6)
                drw3 = em.s3()
                em.bcast3(drw3, tdr[:, 3 * K:4 * K])
                qqw3 = em.s3()
                em.bcast3(qqw3, qqw[:])
                qrv = em.s3()
                qrw = em.s1()
                em.qmul_conj(dr6, drw3, tdr[:, 3 * K:4 * K], qq6, qqw3, qqw[:],
                             qrv[:], qrw[:])
                em.so3_se3(qrv[:], qrw[:], to_r[:])
                S.dma_start(out=rot[s, :, :], in_=to_r[:])

            for t in range(NT):
                edge_tile(t)
            for s in range(NS):
                imu_tile(s)

    nc.finalize()
    _split_excess_waits(nc)
    return nc


# ---------------- host side ----------------

def _soa(x, ns, w, dt=np.float16):
    """[ns*128*w, D] -> [ns, 128, D*w] component-block layout."""
    d = x.shape[1]
    return np.ascontiguousarray(
        x.reshape(ns, P, w, d).transpose(0, 1, 3, 2).reshape(ns, P, d * w)
    ).astype(dt)


def _unsoa(y, d, rows):
    """[ns, 128, d*w] -> [rows, d] float32."""
    ns, _, dw = y.shape
    w = dw // d
    out = y.astype(np.float32).reshape(ns, P, d, w).transpose(0, 1, 3, 2)
    return out.reshape(ns * P * w, d)[:rows]


def kernel(edges, nodes, vels, poses, imu_drots, imu_dtrans, imu_dvels, dts,
           cfg=None, _run=None):
    cfg = cfg or FULL
    E, N, M = cfg["E"], cfg["N"], cfg["M"]
    K, NT, NS, NC = cfg["K"], cfg["NT"], cfg["NS"], cfg["NC"]
    edges = np.asarray(edges)
    nodes = np.asarray(nodes, dtype=np.float32)
    vels = np.asarray(vels, dtype=np.float32)
    poses = np.asarray(poses, dtype=np.float32)
    imu_drots = np.asarray(imu_drots, dtype=np.float32)
    imu_dtrans = np.asarray(imu_dtrans, dtype=np.float32)
    imu_dvels = np.asarray(imu_dvels, dtype=np.float32)
    dts = np.asarray(dts, dtype=np.float32)

    Epc = E // NC
    Epad = P * K * NT
    MW = P * K * NS
    ident7 = np.array([0, 0, 0, 0, 0, 0, 1], np.float32)

    # edge endpoint features (host-side sharding attaches node features to
    # each edge shard; nodes stay replicated conceptually)
    n1 = nodes[edges[:, 0]]
    n2 = nodes[edges[:, 1]]

    # chain padded arrays
    npad_rows = NC * MW + 1 - N
    nodes_pad = np.vstack([nodes, np.tile(ident7, (npad_rows, 1))])
    vels_pad = np.vstack([vels, np.zeros((NC * MW + 1 - N, 3), np.float32)])
    mpad = NC * MW - M
    dr_pad = np.vstack([imu_drots, np.tile(ident7[3:], (mpad, 1))])
    dtr_pad = np.vstack([imu_dtrans, np.zeros((mpad, 3), np.float32)])
    dv_pad = np.vstack([imu_dvels, np.zeros((mpad, 3), np.float32)])
    dts_pad = np.vstack([dts, np.ones((mpad, 1), np.float32)])

    in_maps = []
    for c in range(NC):
        sl = slice(c * Epc, (c + 1) * Epc)

        def etile(x):
            xp = np.tile(ident7, (Epad, 1)).astype(np.float32)
            xp[:Epc] = x[sl]
            return _soa(xp, NT, K)

        base = c * MW
        m = dict(
            en1=etile(n1), en2=etile(n2), eps=etile(poses),
            in0=_soa(nodes_pad[base:base + MW], NS, K),
            in1=_soa(nodes_pad[base + 1:base + MW + 1], NS, K),
            iv0=_soa(vels_pad[base:base + MW], NS, K),
            iv1=_soa(vels_pad[base + 1:base + MW + 1], NS, K),
            idr=_soa(dr_pad[base:base + MW], NS, K),
            idtr=_soa(dtr_pad[base:base + MW], NS, K),
            idv=_soa(dv_pad[base:base + MW], NS, K),
            idts=_soa(dts_pad[base:base + MW], NS, K),
        )
        in_maps.append(m)

    if _run is None:
        from concourse.bass_utils import run_bass_kernel_spmd
        nc = build_nc(cfg)
        res = run_bass_kernel_spmd(nc, in_maps, core_ids=list(range(NC)))
        global LAST_RESULT
        LAST_RESULT = res
        outs = res.results
    else:
        outs = _run(in_maps)

    pgs, adjs, rots, tvls = [], [], [], []
    for c in range(NC):
        o = outs[c]
        pgs.append(_unsoa(o["pg"].reshape(NT, P, 6 * K), 6, Epc))
        adjs.append(_unsoa(o["adj"], 3, MW))
        rots.append(_unsoa(o["rot"], 3, MW))
        tvls.append(_unsoa(o["tvl"], 3, MW))
    pg_full = np.concatenate(pgs, axis=0)
    adj_full = np.concatenate(adjs, axis=0)[:M]
    rot_full = np.concatenate(rots, axis=0)[:M]
    tvl_full = np.concatenate(tvls, axis=0)[:M]
    return np.concatenate([
        pg_full.ravel(), adj_full.ravel(), rot_full.ravel(), tvl_full.ravel()
    ]).astype(np.float32)
